# revision 7
# baseline (speedup 1.0000x reference)
"""Trainium2 Bass kernel for nn_IngredientScannerLoss.

Per row (12 coords = 6 (x,y) pairs):
    delta = output - target
    dist_j = sqrt(dx_j^2 + dy_j^2)
    n_j    = (s0_j*dx_j > 0) + (s1_j*dy_j > 0)   (sign-gated count, 0/1/2)
    f(x)   = ((x+1)^1.2 - 1)*2
    t_j    = [dist, f(dist), f(f(dist))][n_j]
    loss   = sum_j t_j

Data-parallel over 8 NeuronCores: rows split 8 x 500_000, each shard
zero-padded to 501_760 = 128*RT*NT rows.

v4 (v1 343us, v2 287us, v3 289us):
  * Host layout component-major, pair order O2=[2,3,0,1,5,4]: signed
    delta d' = sign*(a-b) via operand-swapped contiguous subtracts
    (Pool 10 RT-units in 5 instrs ~120us, DVE 2 units). Squaring kills
    the sign for dist; gates are (d' > 0). Pairs that can reach n=2
    occupy [0,4RT).
  * DVE work in bf16 (2x tt / 4x ts modes), minimal instruction count
    (~16/tile; measured ~250ns fixed cost per DVE instruction):
    one 12RT square, one 12RT gate ts, select-by-max
      res = max(dist, n*(W0-2), m2*(W1-2)),  m2 = (n>1)
    (valid: f(x) >= 2.4x on x>=0 so d2 >= 2*d1 >= ... >= dist), add
    tree for row sums. No copy_predicated / custom DVE ops (no fast
    modes). Candidates/u-terms alias dead regions of sqf/delta.
  * ACT chain keeps log-space f32 (bf16 there fails tolerance), value
    outputs bf16, single ln+exp table set (patched tables), lt scratch
    shared across tiles (ACT-serial).
  * nt=10 (instr overhead scales with tiles); split pools: inputs
    double-buffered (DMA+Pool stage ~14us/tile each), mid tiles
    triple-buffered. SBUF ~182KB of 208KB.
  Engine busy targets: DMA ~140 (floor), DVE ~146, Pool ~120, ACT ~122.
"""

import numpy as np

import concourse.bacc as bacc
import concourse.bass as bass
import concourse.mybir as mybir
import concourse.tile as tile
from concourse.bass_utils import run_bass_kernel_spmd

P = 128
COLS = 12
NPAIR = 6
B = 4_000_000
N_CORES = 8
ROWS_VALID = B // N_CORES          # 500_000
RT = 392                           # rows per partition per tile
NT = 10                            # tiles per core
ROWS_PC = P * RT * NT              # 501_760 padded rows per core
LN2 = 0.6931471805599453

# original per-coordinate condition signs (reference _SIGNS)
SIGNS = [1.0, 1.0, 1.0, -1.0, -1.0, -1.0, -1.0, 1.0, 0.0, 1.0, 0.0, -1.0]

# pair permutation: block position -> original pair index
PAIR_ORDER = [2, 3, 0, 1, 5, 4]
# component order for the host layout: x-block then y-block
COMP_PERM = [2 * j for j in PAIR_ORDER] + [2 * j + 1 for j in PAIR_ORDER]
# number of leading pair positions that can reach n == 2
NPAIR2 = 4

F32 = mybir.dt.float32
BF16 = mybir.dt.bfloat16
AF = mybir.ActivationFunctionType
ALU = mybir.AluOpType

# subtract runs (start_block, end_block, swap): block b = cols
# [b*RT,(b+1)*RT), sign = SIGNS[COMP_PERM[b]], swap -> b-a.
POOL_RUNS = [
    (0, 2, True),    # x pairs 2,3: sign -1
    (2, 6, False),   # x pairs 0,1 (+1), 5,4 (sign 0, direction free)
    (6, 7, True),    # y pair 2: -1
    (7, 9, False),   # y pairs 3,0: +1
    (11, 12, False), # y pair 4: +1
]
DVE_RUNS = [
    (9, 11, True),   # y pairs 1,5: -1
]

# ---------------------------------------------------------------- act tables
# The stock table-load pass resolves Exp -> exp_and_others and
# Ln -> natural_log, reloading ACT tables on every Ln<->Exp switch
# (~1.3us each). Restrict ln/exp membership to sets that hold BOTH so
# every activation resolves to natural_log_exp_and_others and the load
# hoists to one per kernel. Dict order (act_func_set_id) is preserved.

_GAT_REAL = None


def _gat_lnexp(arch):
    global _GAT_REAL
    from concourse.hw_specs import get_activation_tables

    if _GAT_REAL is None:
        _GAT_REAL = get_activation_tables
    tabs = _GAT_REAL(arch)
    out = {}
    for name, funcs in tabs.items():
        fs = set(funcs)
        if not (AF.Ln in fs and AF.Exp in fs):
            fs.discard(AF.Ln)
            fs.discard(AF.Exp)
        out[name] = fs
    return out


def _patch_act_tables():
    if bacc.get_activation_tables is not _gat_lnexp:
        global _GAT_REAL
        _GAT_REAL = bacc.get_activation_tables
        bacc.get_activation_tables = _gat_lnexp


# ---------------------------------------------------------------- bass build


def build_nc(rt: int = RT, nt: int = NT):
    """Single-core SPMD program: inputs [nt, P, 12*rt] comp-major f32."""
    _patch_act_tables()
    nc = bacc.Bacc("TRN2", debug=False, target_bir_lowering=False,
                   num_devices=N_CORES)
    # activation biases need registered const APs (only 0.0/1.0 ship)
    for cv in (-1.0, LN2):
        if (F32, cv) not in nc.const_aps.aps:
            ct = nc.alloc_sbuf_tensor(f"const-f32-{cv}", [P, 1], F32)
            nc.gpsimd.memset(ct.ap(), cv)
            nc.const_aps.aps[(F32, cv)] = ct.ap()
    nc.all_engine_barrier()
    w12 = rt * COLS
    w6 = rt * NPAIR
    w4 = rt * NPAIR2
    w3 = rt * 3
    a = nc.dram_tensor("output", [nt, P, w12], F32, kind="ExternalInput").ap()
    b = nc.dram_tensor("target", [nt, P, w12], F32, kind="ExternalInput").ap()
    o = nc.dram_tensor("loss", [nt, P, rt], F32, kind="ExternalOutput").ap()

    with tile.TileContext(nc) as tc:
        with tc.tile_pool(name="loc", bufs=1) as lpool, \
             tc.tile_pool(name="loc2", bufs=2) as lpool2, \
             tc.tile_pool(name="pin", bufs=2) as pin, \
             tc.tile_pool(name="pmid", bufs=3) as pmid:
            # DVE scratch. sqf is produce/consume-adjacent in the DVE
            # stream (1 buf); g/m2 span the 1-tile software-pipeline
            # skew (2 bufs). lt is ACT-serial (1 buf).
            lt = lpool.tile([P, w6], F32, tag="lt")        # ACT only
            sqf = lpool.tile([P, w12], BF16, tag="sqf")    # DVE only
            st = {}

            def emit_a1(i):
                """DMA-in + signed delta d' (Pool runs + one DVE run)."""
                ta = pin.tile([P, w12], F32, tag="ta")
                nc.sync.dma_start(out=ta[:], in_=a[i])
                tb = pin.tile([P, w12], F32, tag="tb")
                nc.sync.dma_start(out=tb[:], in_=b[i])
                delta = pmid.tile([P, w12], BF16, tag="delta")
                for (lo, hi, swap) in POOL_RUNS:
                    xs = slice(lo * rt, hi * rt)
                    src0, src1 = (tb, ta) if swap else (ta, tb)
                    nc.gpsimd.tensor_tensor(
                        delta[:, xs], src0[:, xs], src1[:, xs], ALU.subtract)
                for (lo, hi, swap) in DVE_RUNS:
                    xs = slice(lo * rt, hi * rt)
                    src0, src1 = (tb, ta) if swap else (ta, tb)
                    nc.vector.tensor_tensor(
                        delta[:, xs], src0[:, xs], src1[:, xs], ALU.subtract)
                st[i] = {"delta": delta}

            def emit_a2(i):
                """Squares + gates on DVE: s, g, n4, m2."""
                delta = st[i]["delta"]
                s = pmid.tile([P, w6], BF16, tag="s")
                nc.vector.tensor_tensor(sqf[:], delta[:], delta[:], ALU.mult)
                nc.vector.tensor_tensor(s[:], sqf[:, 0:w6], sqf[:, w6:w12],
                                        ALU.add)
                g = lpool2.tile([P, w12], BF16, tag="g")
                nc.vector.tensor_scalar(g[:], delta[:], 0.0, None, ALU.is_gt)
                n4 = g[:, 0:w4]
                nc.vector.tensor_tensor(n4, n4, g[:, w6:w6 + w4], ALU.add)
                m2 = lpool2.tile([P, w4], BF16, tag="m2")
                nc.vector.tensor_scalar(m2[:], n4, 1.0, None, ALU.is_gt)
                st[i].update(s=s, g=g, m2=m2)

            def emit_act(i):
                """ACT chain, one ln+exp table set; log-space f32 in lt:
                ls = ln(s); dist = exp(0.5*ls); t = ln(dist+1);
                W0 = exp(1.2*t + ln2) = d1+2; t2 = ln(W0-1);
                W1 = exp(1.2*t2 + ln2) = d2+2 on [0,4RT)."""
                s = st[i]["s"]
                nc.scalar.activation(lt[:], s[:], AF.Ln)
                dist = pmid.tile([P, w6], BF16, tag="dist")
                nc.scalar.activation(dist[:], lt[:], AF.Exp, scale=0.5)
                nc.scalar.activation(lt[:], dist[:], AF.Ln, bias=1.0)
                W0 = pmid.tile([P, w6], BF16, tag="W0")
                nc.scalar.activation(W0[:], lt[:], AF.Exp, scale=1.2,
                                     bias=LN2)
                t2 = lt[:, 0:w4]
                nc.scalar.activation(t2, W0[:, 0:w4], AF.Ln, bias=-1.0)
                W1 = s[:, 0:w4]  # s dead after ln(s)
                nc.scalar.activation(W1, t2, AF.Exp, scale=1.2, bias=LN2)
                st[i].update(dist=dist, W0=W0, W1=W1)

            def emit_b(i):
                """Select-by-max res = max(dist, n*(W0-2), m2*(W1-2)),
                row-sum tree, DMA-out. d1/d2 -> sqf regions (dead),
                u1/u2 -> delta[w6:] (dy dead), res -> delta[0:w6]."""
                v = st.pop(i)
                delta, g, m2 = v["delta"], v["g"], v["m2"]
                dist, W0, W1 = v["dist"], v["W0"], v["W1"]
                n4 = g[:, 0:w4]
                nc.vector.tensor_scalar(sqf[:, 0:w6], W0[:], 2.0, None,
                                        ALU.subtract)
                nc.vector.tensor_tensor(delta[:, w6:w6 + w4], sqf[:, 0:w4],
                                        n4, ALU.mult)
                nc.vector.tensor_tensor(delta[:, w6 + w4:w12],
                                        sqf[:, w4:w6],
                                        g[:, w6 + w4:w12], ALU.mult)
                nc.vector.tensor_tensor(delta[:, 0:w6], dist[:],
                                        delta[:, w6:w12], ALU.max)
                nc.vector.tensor_scalar(sqf[:, w6:w6 + w4], W1, 2.0, None,
                                        ALU.subtract)
                nc.vector.tensor_tensor(delta[:, w6:w6 + w4],
                                        sqf[:, w6:w6 + w4], m2[:], ALU.mult)
                nc.vector.tensor_tensor(delta[:, 0:w4], delta[:, 0:w4],
                                        delta[:, w6:w6 + w4], ALU.max)
                nc.vector.tensor_tensor(delta[:, 0:w3], delta[:, 0:w3],
                                        delta[:, w3:2 * w3], ALU.add)
                nc.vector.tensor_tensor(delta[:, 0:rt], delta[:, 0:rt],
                                        delta[:, rt:2 * rt], ALU.add)
                ot = pmid.tile([P, rt], F32, tag="ot")
                nc.vector.tensor_tensor(ot[:], delta[:, 0:rt],
                                        delta[:, 2 * rt:w3], ALU.add)
                nc.sync.dma_start(out=o[i], in_=ot[:])

            # software-pipelined emission, 1-tile skew: DVE queue order
            # is [.. sub(i+1), B(i), A2(i+1) ..] so tile i+1's pre-ACT
            # work fills the DVE stall while ACT(i) produces W0/W1.
            emit_a1(0)
            emit_a2(0)
            for i in range(nt):
                emit_act(i)
                if i + 1 < nt:
                    emit_a1(i + 1)
                emit_b(i)
                if i + 1 < nt:
                    emit_a2(i + 1)
    nc.compile()
    return nc


_NC_CACHE: dict = {}


def _get_nc(rt: int = RT, nt: int = NT):
    key = (rt, nt)
    if key not in _NC_CACHE:
        _NC_CACHE[key] = build_nc(rt, nt)
    return _NC_CACHE[key]


# ---------------------------------------------------------------- host shard


def make_in_maps(a: np.ndarray, b: np.ndarray, rt: int = RT, nt: int = NT):
    """Shard + component-major permute: [B,12] -> 8 x [nt, P, 12*rt]."""
    rows_pc = P * rt * nt
    perm = np.asarray(COMP_PERM, dtype=np.int64)

    def shard(x):
        sh = np.zeros((N_CORES, rows_pc, COLS), dtype=np.float32)
        sh[:, :ROWS_VALID, :] = x.reshape(N_CORES, ROWS_VALID, COLS)[..., perm]
        # [C, nt, P, rt, 12] -> [C, nt, P, 12, rt] -> [C, nt, P, 12*rt]
        sh = sh.reshape(N_CORES, nt, P, rt, COLS)
        sh = np.ascontiguousarray(sh.transpose(0, 1, 2, 4, 3))
        return sh.reshape(N_CORES, nt, P, COLS * rt)

    a_sh = shard(a)
    b_sh = shard(b)
    return [
        {"output": a_sh[c], "target": b_sh[c]} for c in range(N_CORES)
    ]


# ---------------------------------------------------------------- entrypoint


def kernel(output, target):
    a = np.asarray(output, dtype=np.float32)
    b = np.asarray(target, dtype=np.float32)
    assert a.shape == (B, COLS) and b.shape == (B, COLS)

    nc = _get_nc()
    in_maps = make_in_maps(a, b)
    r = run_bass_kernel_spmd(nc, in_maps, list(range(N_CORES)))
    out = np.empty((N_CORES, ROWS_VALID), dtype=np.float32)
    for c in range(N_CORES):
        loss = r.results[c]["loss"].reshape(NT * P * RT)
        out[c] = loss[:ROWS_VALID]
    return out.reshape(B)


# revision 10
# speedup vs baseline: 1.0631x; 1.0631x over previous
"""Trainium2 Bass kernel for nn_IngredientScannerLoss.

Per row (12 coords = 6 (x,y) pairs):
    delta = output - target
    dist_j = sqrt(dx_j^2 + dy_j^2)
    n_j    = (s0_j*dx_j > 0) + (s1_j*dy_j > 0)   (sign-gated count, 0/1/2)
    f(x)   = ((x+1)^1.2 - 1)*2
    t_j    = [dist, f(dist), f(f(dist))][n_j]
    loss   = sum_j t_j

Data-parallel over 8 NeuronCores: rows split 8 x 500_000, each shard
zero-padded to 501_760 = 128*RT*NT rows.

v4 (v1 343us, v2 287us, v3 289us):
  * Host layout component-major, pair order O2=[2,3,0,1,5,4]: signed
    delta d' = sign*(a-b) via operand-swapped contiguous subtracts
    (Pool 10 RT-units in 5 instrs ~120us, DVE 2 units). Squaring kills
    the sign for dist; gates are (d' > 0). Pairs that can reach n=2
    occupy [0,4RT).
  * DVE work in bf16 (2x tt / 4x ts modes), minimal instruction count
    (~16/tile; measured ~250ns fixed cost per DVE instruction):
    one 12RT square, one 12RT gate ts, select-by-max
      res = max(dist, n*(W0-2), m2*(W1-2)),  m2 = (n>1)
    (valid: f(x) >= 2.4x on x>=0 so d2 >= 2*d1 >= ... >= dist), add
    tree for row sums. No copy_predicated / custom DVE ops (no fast
    modes). Candidates/u-terms alias dead regions of sqf/delta.
  * ACT chain keeps log-space f32 (bf16 there fails tolerance), value
    outputs bf16, single ln+exp table set (patched tables), lt scratch
    shared across tiles (ACT-serial).
  * nt=10 (instr overhead scales with tiles); split pools: inputs
    double-buffered (DMA+Pool stage ~14us/tile each), mid tiles
    triple-buffered. SBUF ~182KB of 208KB.
  Engine busy targets: DMA ~140 (floor), DVE ~146, Pool ~120, ACT ~122.
"""

import numpy as np

import concourse.bacc as bacc
import concourse.bass as bass
import concourse.mybir as mybir
import concourse.tile as tile
from concourse.bass_utils import run_bass_kernel_spmd

P = 128
COLS = 12
NPAIR = 6
B = 4_000_000
N_CORES = 8
ROWS_VALID = B // N_CORES          # 500_000
RT = 392                           # rows per partition per tile
NT = 10                            # tiles per core
ROWS_PC = P * RT * NT              # 501_760 padded rows per core
LN2 = 0.6931471805599453

# original per-coordinate condition signs (reference _SIGNS)
SIGNS = [1.0, 1.0, 1.0, -1.0, -1.0, -1.0, -1.0, 1.0, 0.0, 1.0, 0.0, -1.0]

# pair permutation: block position -> original pair index
PAIR_ORDER = [2, 3, 0, 1, 5, 4]
# component order for the host layout: x-block then y-block
COMP_PERM = [2 * j for j in PAIR_ORDER] + [2 * j + 1 for j in PAIR_ORDER]
# number of leading pair positions that can reach n == 2
NPAIR2 = 4

F32 = mybir.dt.float32
BF16 = mybir.dt.bfloat16
AF = mybir.ActivationFunctionType
ALU = mybir.AluOpType

# subtract runs (start_block, end_block, swap): block b = cols
# [b*RT,(b+1)*RT), sign = SIGNS[COMP_PERM[b]], swap -> b-a.
POOL_RUNS = [
    (0, 2, True),    # x pairs 2,3: sign -1
    (2, 6, False),   # x pairs 0,1 (+1), 5,4 (sign 0, direction free)
    (6, 7, True),    # y pair 2: -1
    (7, 9, False),   # y pairs 3,0: +1
    (9, 10, True),   # y pair 1: -1   (1-unit runs: Pool slows down
    (10, 11, True),  # y pair 5: -1    nonlinearly on wider ones)
    (11, 12, False), # y pair 4: +1
]
DVE_RUNS = []

# ---------------------------------------------------------------- act tables
# The stock table-load pass resolves Exp -> exp_and_others and
# Ln -> natural_log, reloading ACT tables on every Ln<->Exp switch
# (~1.3us each). Restrict ln/exp membership to sets that hold BOTH so
# every activation resolves to natural_log_exp_and_others and the load
# hoists to one per kernel. Dict order (act_func_set_id) is preserved.

_GAT_REAL = None


def _gat_lnexp(arch):
    global _GAT_REAL
    from concourse.hw_specs import get_activation_tables

    if _GAT_REAL is None:
        _GAT_REAL = get_activation_tables
    tabs = _GAT_REAL(arch)
    out = {}
    for name, funcs in tabs.items():
        fs = set(funcs)
        if not (AF.Ln in fs and AF.Exp in fs):
            fs.discard(AF.Ln)
            fs.discard(AF.Exp)
        out[name] = fs
    return out


def _patch_act_tables():
    if bacc.get_activation_tables is not _gat_lnexp:
        global _GAT_REAL
        _GAT_REAL = bacc.get_activation_tables
        bacc.get_activation_tables = _gat_lnexp


# ---------------------------------------------------------------- bass build


def build_nc(rt: int = RT, nt: int = NT):
    """Single-core SPMD program: inputs [nt, P, 12*rt] comp-major f32."""
    _patch_act_tables()
    nc = bacc.Bacc("TRN2", debug=False, target_bir_lowering=False,
                   num_devices=N_CORES)
    # activation biases need registered const APs (only 0.0/1.0 ship)
    for cv in (-1.0, LN2):
        if (F32, cv) not in nc.const_aps.aps:
            ct = nc.alloc_sbuf_tensor(f"const-f32-{cv}", [P, 1], F32)
            nc.gpsimd.memset(ct.ap(), cv)
            nc.const_aps.aps[(F32, cv)] = ct.ap()
    nc.all_engine_barrier()
    w12 = rt * COLS
    w6 = rt * NPAIR
    w4 = rt * NPAIR2
    w3 = rt * 3
    a = nc.dram_tensor("output", [nt, P, w12], F32, kind="ExternalInput").ap()
    b = nc.dram_tensor("target", [nt, P, w12], F32, kind="ExternalInput").ap()
    o = nc.dram_tensor("loss", [nt, P, rt], F32, kind="ExternalOutput").ap()

    with tile.TileContext(nc) as tc:
        with tc.tile_pool(name="loc", bufs=1) as lpool, \
             tc.tile_pool(name="loc2", bufs=2) as lpool2, \
             tc.tile_pool(name="pin", bufs=2) as pin, \
             tc.tile_pool(name="pmid", bufs=3) as pmid:
            # DVE scratch. sqf is produce/consume-adjacent in the DVE
            # stream (1 buf); g/m2 span the 1-tile software-pipeline
            # skew (2 bufs). lt is ACT-serial (1 buf).
            lt = lpool.tile([P, w6], F32, tag="lt")        # ACT only
            sqf = lpool.tile([P, w12], BF16, tag="sqf")    # DVE only
            st = {}

            def emit_a1(i):
                """DMA-in + signed delta d' (Pool runs + one DVE run)."""
                ta = pin.tile([P, w12], F32, tag="ta")
                nc.sync.dma_start(out=ta[:], in_=a[i])
                tb = pin.tile([P, w12], F32, tag="tb")
                nc.sync.dma_start(out=tb[:], in_=b[i])
                delta = pmid.tile([P, w12], BF16, tag="delta")
                for (lo, hi, swap) in POOL_RUNS:
                    xs = slice(lo * rt, hi * rt)
                    src0, src1 = (tb, ta) if swap else (ta, tb)
                    nc.gpsimd.tensor_tensor(
                        delta[:, xs], src0[:, xs], src1[:, xs], ALU.subtract)
                for (lo, hi, swap) in DVE_RUNS:
                    xs = slice(lo * rt, hi * rt)
                    src0, src1 = (tb, ta) if swap else (ta, tb)
                    nc.vector.tensor_tensor(
                        delta[:, xs], src0[:, xs], src1[:, xs], ALU.subtract)
                st[i] = {"delta": delta}

            def emit_a2(i):
                """Squares + gates on DVE: s, g, n4, m2."""
                delta = st[i]["delta"]
                s = pmid.tile([P, w6], BF16, tag="s")
                nc.vector.tensor_tensor(sqf[:], delta[:], delta[:], ALU.mult)
                nc.vector.tensor_tensor(s[:], sqf[:, 0:w6], sqf[:, w6:w12],
                                        ALU.add)
                g = lpool2.tile([P, w12], BF16, tag="g")
                nc.vector.tensor_scalar(g[:], delta[:], 0.0, None, ALU.is_gt)
                n4 = g[:, 0:w4]
                nc.vector.tensor_tensor(n4, n4, g[:, w6:w6 + w4], ALU.add)
                m2 = lpool2.tile([P, w4], BF16, tag="m2")
                nc.vector.tensor_scalar(m2[:], n4, 1.0, None, ALU.is_gt)
                st[i].update(s=s, g=g, m2=m2)

            def emit_act(i):
                """ACT chain, one ln+exp table set; log-space f32 in lt:
                ls = ln(s); dist = exp(0.5*ls); t = ln(dist+1);
                W0 = exp(1.2*t + ln2) = d1+2; t2 = ln(W0-1);
                W1 = exp(1.2*t2 + ln2) = d2+2 on [0,4RT)."""
                s = st[i]["s"]
                nc.scalar.activation(lt[:], s[:], AF.Ln)
                dist = pmid.tile([P, w6], BF16, tag="dist")
                nc.scalar.activation(dist[:], lt[:], AF.Exp, scale=0.5)
                nc.scalar.activation(lt[:], dist[:], AF.Ln, bias=1.0)
                W0 = pmid.tile([P, w6], BF16, tag="W0")
                nc.scalar.activation(W0[:], lt[:], AF.Exp, scale=1.2,
                                     bias=LN2)
                t2 = lt[:, 0:w4]
                nc.scalar.activation(t2, W0[:, 0:w4], AF.Ln, bias=-1.0)
                W1 = s[:, 0:w4]  # s dead after ln(s)
                nc.scalar.activation(W1, t2, AF.Exp, scale=1.2, bias=LN2)
                st[i].update(dist=dist, W0=W0, W1=W1)

            def emit_b(i):
                """Select-by-max res = max(dist, n*(W0-2), m2*(W1-2)),
                row-sum tree, DMA-out. d1/d2 -> sqf regions (dead),
                u1/u2 -> delta[w6:] (dy dead), res -> delta[0:w6]."""
                v = st.pop(i)
                delta, g, m2 = v["delta"], v["g"], v["m2"]
                dist, W0, W1 = v["dist"], v["W0"], v["W1"]
                n4 = g[:, 0:w4]
                nc.vector.tensor_scalar(sqf[:, 0:w6], W0[:], 2.0, None,
                                        ALU.subtract)
                nc.vector.tensor_tensor(delta[:, w6:w6 + w4], sqf[:, 0:w4],
                                        n4, ALU.mult)
                nc.vector.tensor_tensor(delta[:, w6 + w4:w12],
                                        sqf[:, w4:w6],
                                        g[:, w6 + w4:w12], ALU.mult)
                nc.vector.tensor_tensor(delta[:, 0:w6], dist[:],
                                        delta[:, w6:w12], ALU.max)
                nc.vector.tensor_scalar(sqf[:, w6:w6 + w4], W1, 2.0, None,
                                        ALU.subtract)
                nc.vector.tensor_tensor(delta[:, w6:w6 + w4],
                                        sqf[:, w6:w6 + w4], m2[:], ALU.mult)
                nc.vector.tensor_tensor(delta[:, 0:w4], delta[:, 0:w4],
                                        delta[:, w6:w6 + w4], ALU.max)
                nc.vector.tensor_tensor(delta[:, 0:w3], delta[:, 0:w3],
                                        delta[:, w3:2 * w3], ALU.add)
                nc.vector.tensor_tensor(delta[:, 0:rt], delta[:, 0:rt],
                                        delta[:, rt:2 * rt], ALU.add)
                ot = pmid.tile([P, rt], F32, tag="ot")
                nc.vector.tensor_tensor(ot[:], delta[:, 0:rt],
                                        delta[:, 2 * rt:w3], ALU.add)
                nc.sync.dma_start(out=o[i], in_=ot[:])

            # software-pipelined emission, 1-tile skew with the fill
            # work AHEAD of the stalling stage in the in-order DVE
            # queue: [.. A2(i+1), B(i) ..] -- B(i)'s d1 must wait for
            # ACT(i) stage 4 (W0), and A2(i+1) runs during that wait.
            emit_a1(0)
            emit_a2(0)
            for i in range(nt):
                emit_act(i)
                if i + 1 < nt:
                    emit_a1(i + 1)
                    emit_a2(i + 1)
                emit_b(i)
    nc.compile()
    return nc


_NC_CACHE: dict = {}


def _get_nc(rt: int = RT, nt: int = NT):
    key = (rt, nt)
    if key not in _NC_CACHE:
        _NC_CACHE[key] = build_nc(rt, nt)
    return _NC_CACHE[key]


# ---------------------------------------------------------------- host shard


def make_in_maps(a: np.ndarray, b: np.ndarray, rt: int = RT, nt: int = NT):
    """Shard + component-major permute: [B,12] -> 8 x [nt, P, 12*rt]."""
    rows_pc = P * rt * nt
    perm = np.asarray(COMP_PERM, dtype=np.int64)

    def shard(x):
        sh = np.zeros((N_CORES, rows_pc, COLS), dtype=np.float32)
        sh[:, :ROWS_VALID, :] = x.reshape(N_CORES, ROWS_VALID, COLS)[..., perm]
        # [C, nt, P, rt, 12] -> [C, nt, P, 12, rt] -> [C, nt, P, 12*rt]
        sh = sh.reshape(N_CORES, nt, P, rt, COLS)
        sh = np.ascontiguousarray(sh.transpose(0, 1, 2, 4, 3))
        return sh.reshape(N_CORES, nt, P, COLS * rt)

    a_sh = shard(a)
    b_sh = shard(b)
    return [
        {"output": a_sh[c], "target": b_sh[c]} for c in range(N_CORES)
    ]


# ---------------------------------------------------------------- entrypoint


def kernel(output, target):
    a = np.asarray(output, dtype=np.float32)
    b = np.asarray(target, dtype=np.float32)
    assert a.shape == (B, COLS) and b.shape == (B, COLS)

    nc = _get_nc()
    in_maps = make_in_maps(a, b)
    r = run_bass_kernel_spmd(nc, in_maps, list(range(N_CORES)))
    out = np.empty((N_CORES, ROWS_VALID), dtype=np.float32)
    for c in range(N_CORES):
        loss = r.results[c]["loss"].reshape(NT * P * RT)
        out[c] = loss[:ROWS_VALID]
    return out.reshape(B)


# revision 12
# speedup vs baseline: 1.0899x; 1.0252x over previous
"""Trainium2 Bass kernel for nn_IngredientScannerLoss.

Per row (12 coords = 6 (x,y) pairs):
    delta = output - target
    dist_j = sqrt(dx_j^2 + dy_j^2)
    n_j    = (s0_j*dx_j > 0) + (s1_j*dy_j > 0)   (sign-gated count, 0/1/2)
    f(x)   = ((x+1)^1.2 - 1)*2
    t_j    = [dist, f(dist), f(f(dist))][n_j]
    loss   = sum_j t_j

Data-parallel over 8 NeuronCores: rows split 8 x 500_000, each shard
zero-padded to 501_760 = 128*RT*NT rows.

v4 (v1 343us, v2 287us, v3 289us):
  * Host layout component-major, pair order O2=[2,3,0,1,5,4]: signed
    delta d' = sign*(a-b) via operand-swapped contiguous subtracts
    (Pool 10 RT-units in 5 instrs ~120us, DVE 2 units). Squaring kills
    the sign for dist; gates are (d' > 0). Pairs that can reach n=2
    occupy [0,4RT).
  * DVE work in bf16 (2x tt / 4x ts modes), minimal instruction count
    (~16/tile; measured ~250ns fixed cost per DVE instruction):
    one 12RT square, one 12RT gate ts, select-by-max
      res = max(dist, n*(W0-2), m2*(W1-2)),  m2 = (n>1)
    (valid: f(x) >= 2.4x on x>=0 so d2 >= 2*d1 >= ... >= dist), add
    tree for row sums. No copy_predicated / custom DVE ops (no fast
    modes). Candidates/u-terms alias dead regions of sqf/delta.
  * ACT chain keeps log-space f32 (bf16 there fails tolerance), value
    outputs bf16, single ln+exp table set (patched tables), lt scratch
    shared across tiles (ACT-serial).
  * nt=10 (instr overhead scales with tiles); split pools: inputs
    double-buffered (DMA+Pool stage ~14us/tile each), mid tiles
    triple-buffered. SBUF ~182KB of 208KB.
  Engine busy targets: DMA ~140 (floor), DVE ~146, Pool ~120, ACT ~122.
"""

import numpy as np

import concourse.bacc as bacc
import concourse.bass as bass
import concourse.mybir as mybir
import concourse.tile as tile
from concourse.bass_utils import run_bass_kernel_spmd

P = 128
COLS = 12
NPAIR = 6
B = 4_000_000
N_CORES = 8
ROWS_VALID = B // N_CORES          # 500_000
RT = 392                           # rows per partition per tile
NT = 10                            # tiles per core
ROWS_PC = P * RT * NT              # 501_760 padded rows per core
LN2 = 0.6931471805599453

# original per-coordinate condition signs (reference _SIGNS)
SIGNS = [1.0, 1.0, 1.0, -1.0, -1.0, -1.0, -1.0, 1.0, 0.0, 1.0, 0.0, -1.0]

# pair permutation: block position -> original pair index
PAIR_ORDER = [2, 3, 0, 1, 5, 4]
# component order for the host layout: x-block then y-block
COMP_PERM = [2 * j for j in PAIR_ORDER] + [2 * j + 1 for j in PAIR_ORDER]
# number of leading pair positions that can reach n == 2
NPAIR2 = 4

F32 = mybir.dt.float32
BF16 = mybir.dt.bfloat16
AF = mybir.ActivationFunctionType
ALU = mybir.AluOpType

# subtract runs (start_block, end_block, swap): block b = cols
# [b*RT,(b+1)*RT), sign = SIGNS[COMP_PERM[b]], swap -> b-a.
POOL_RUNS = [
    (0, 2, True),    # x pairs 2,3: sign -1
    (2, 6, False),   # x pairs 0,1 (+1), 5,4 (sign 0, direction free)
    (6, 7, True),    # y pair 2: -1
    (7, 9, False),   # y pairs 3,0: +1
    (9, 10, True),   # y pair 1: -1   (1-unit runs: Pool slows down
    (10, 11, True),  # y pair 5: -1    nonlinearly on wider ones)
    (11, 12, False), # y pair 4: +1
]
DVE_RUNS = []

# ---------------------------------------------------------------- act tables
# The stock table-load pass resolves Exp -> exp_and_others and
# Ln -> natural_log, reloading ACT tables on every Ln<->Exp switch
# (~1.3us each). Restrict ln/exp membership to sets that hold BOTH so
# every activation resolves to natural_log_exp_and_others and the load
# hoists to one per kernel. Dict order (act_func_set_id) is preserved.

_GAT_REAL = None


def _gat_lnexp(arch):
    global _GAT_REAL
    from concourse.hw_specs import get_activation_tables

    if _GAT_REAL is None:
        _GAT_REAL = get_activation_tables
    tabs = _GAT_REAL(arch)
    out = {}
    for name, funcs in tabs.items():
        fs = set(funcs)
        if not (AF.Ln in fs and AF.Exp in fs):
            fs.discard(AF.Ln)
            fs.discard(AF.Exp)
        out[name] = fs
    return out


def _patch_act_tables():
    if bacc.get_activation_tables is not _gat_lnexp:
        global _GAT_REAL
        _GAT_REAL = bacc.get_activation_tables
        bacc.get_activation_tables = _gat_lnexp


# ---------------------------------------------------------------- bass build


def build_nc(rt: int = RT, nt: int = NT):
    """Single-core SPMD program: inputs [nt, P, 12*rt] comp-major f32."""
    _patch_act_tables()
    nc = bacc.Bacc("TRN2", debug=False, target_bir_lowering=False,
                   num_devices=N_CORES)
    # activation biases need registered const APs (only 0.0/1.0 ship)
    for cv in (-1.0, LN2):
        if (F32, cv) not in nc.const_aps.aps:
            ct = nc.alloc_sbuf_tensor(f"const-f32-{cv}", [P, 1], F32)
            nc.gpsimd.memset(ct.ap(), cv)
            nc.const_aps.aps[(F32, cv)] = ct.ap()
    nc.all_engine_barrier()
    w12 = rt * COLS
    w6 = rt * NPAIR
    w4 = rt * NPAIR2
    w3 = rt * 3
    a = nc.dram_tensor("output", [nt, P, w12], F32, kind="ExternalInput").ap()
    b = nc.dram_tensor("target", [nt, P, w12], F32, kind="ExternalInput").ap()
    o = nc.dram_tensor("loss", [nt, P, rt], F32, kind="ExternalOutput").ap()

    with tile.TileContext(nc) as tc:
        with tc.tile_pool(name="loc", bufs=1) as lpool, \
             tc.tile_pool(name="loc2", bufs=2) as lpool2, \
             tc.tile_pool(name="pin", bufs=2) as pin, \
             tc.tile_pool(name="pmid", bufs=3) as pmid:
            # DVE scratch. sqf is produce/consume-adjacent in the DVE
            # stream (1 buf); g/m2 span the 1-tile software-pipeline
            # skew (2 bufs). lt is ACT-serial (1 buf).
            lt = lpool.tile([P, w6], F32, tag="lt")        # ACT only
            sqf = lpool.tile([P, w12], BF16, tag="sqf")    # DVE only
            st = {}

            def emit_a1(i):
                """DMA-in + signed delta d' (Pool runs + one DVE run)."""
                ta = pin.tile([P, w12], F32, tag="ta")
                nc.sync.dma_start(out=ta[:], in_=a[i])
                tb = pin.tile([P, w12], F32, tag="tb")
                nc.sync.dma_start(out=tb[:], in_=b[i])
                delta = pmid.tile([P, w12], BF16, tag="delta")
                for (lo, hi, swap) in POOL_RUNS:
                    xs = slice(lo * rt, hi * rt)
                    src0, src1 = (tb, ta) if swap else (ta, tb)
                    nc.gpsimd.tensor_tensor(
                        delta[:, xs], src0[:, xs], src1[:, xs], ALU.subtract)
                for (lo, hi, swap) in DVE_RUNS:
                    xs = slice(lo * rt, hi * rt)
                    src0, src1 = (tb, ta) if swap else (ta, tb)
                    nc.vector.tensor_tensor(
                        delta[:, xs], src0[:, xs], src1[:, xs], ALU.subtract)
                st[i] = {"delta": delta}

            def emit_a2(i):
                """Squares + gates on DVE: s, g, n4, m2."""
                delta = st[i]["delta"]
                s = pmid.tile([P, w6], BF16, tag="s")
                nc.vector.tensor_tensor(sqf[:], delta[:], delta[:], ALU.mult)
                nc.vector.tensor_tensor(s[:], sqf[:, 0:w6], sqf[:, w6:w12],
                                        ALU.add)
                g = lpool2.tile([P, w12], BF16, tag="g")
                nc.vector.tensor_scalar(g[:], delta[:], 0.0, None, ALU.is_gt)
                n4 = g[:, 0:w4]
                nc.vector.tensor_tensor(n4, n4, g[:, w6:w6 + w4], ALU.add)
                m2 = lpool2.tile([P, w4], BF16, tag="m2")
                nc.vector.tensor_scalar(m2[:], n4, 1.0, None, ALU.is_gt)
                st[i].update(s=s, g=g, m2=m2)

            def emit_act(i):
                """ACT chain, one ln+exp table set; log-space f32 in lt:
                ls = ln(s); dist = exp(0.5*ls); t = ln(dist+1);
                W0 = exp(1.2*t + ln2) = d1+2; t2 = ln(W0-1);
                W1 = exp(1.2*t2 + ln2) = d2+2 on [0,4RT)."""
                s = st[i]["s"]
                nc.scalar.activation(lt[:], s[:], AF.Ln)
                dist = pmid.tile([P, w6], BF16, tag="dist")
                nc.scalar.activation(dist[:], lt[:], AF.Exp, scale=0.5)
                nc.scalar.activation(lt[:], dist[:], AF.Ln, bias=1.0)
                W0 = pmid.tile([P, w6], BF16, tag="W0")
                nc.scalar.activation(W0[:], lt[:], AF.Exp, scale=1.2,
                                     bias=LN2)
                t2 = lt[:, 0:w4]
                nc.scalar.activation(t2, W0[:, 0:w4], AF.Ln, bias=-1.0)
                W1 = s[:, 0:w4]  # s dead after ln(s)
                nc.scalar.activation(W1, t2, AF.Exp, scale=1.2, bias=LN2)
                # candidates d1 = W0-2, d2 = W1-2 in place on ACT
                # (Copy applies scale/bias; ACT has slack, DVE is paced)
                nc.scalar.activation(W0[:], W0[:], AF.Copy, bias=-2.0)
                nc.scalar.activation(W1, W1, AF.Copy, bias=-2.0)
                st[i].update(dist=dist, W0=W0, W1=W1)

            def emit_b(i):
                """Select-by-max res = max(dist, n*(W0-2), m2*(W1-2)),
                row-sum tree, DMA-out. d1/d2 -> sqf regions (dead),
                u1/u2 -> delta[w6:] (dy dead), res -> delta[0:w6]."""
                v = st.pop(i)
                delta, g, m2 = v["delta"], v["g"], v["m2"]
                dist, W0, W1 = v["dist"], v["W0"], v["W1"]
                n4 = g[:, 0:w4]
                nc.vector.tensor_tensor(delta[:, w6:w6 + w4], W0[:, 0:w4],
                                        n4, ALU.mult)
                nc.vector.tensor_tensor(delta[:, w6 + w4:w12],
                                        W0[:, w4:w6],
                                        g[:, w6 + w4:w12], ALU.mult)
                nc.vector.tensor_tensor(delta[:, 0:w6], dist[:],
                                        delta[:, w6:w12], ALU.max)
                nc.vector.tensor_tensor(delta[:, w6:w6 + w4],
                                        W1, m2[:], ALU.mult)
                nc.vector.tensor_tensor(delta[:, 0:w4], delta[:, 0:w4],
                                        delta[:, w6:w6 + w4], ALU.max)
                nc.vector.tensor_tensor(delta[:, 0:w3], delta[:, 0:w3],
                                        delta[:, w3:2 * w3], ALU.add)
                nc.vector.tensor_tensor(delta[:, 0:rt], delta[:, 0:rt],
                                        delta[:, rt:2 * rt], ALU.add)
                ot = pmid.tile([P, rt], F32, tag="ot")
                nc.vector.tensor_tensor(ot[:], delta[:, 0:rt],
                                        delta[:, 2 * rt:w3], ALU.add)
                nc.sync.dma_start(out=o[i], in_=ot[:])

            # software-pipelined emission, 1-tile skew with the fill
            # work AHEAD of the stalling stage in the in-order DVE
            # queue: [.. A2(i+1), B(i) ..] -- B(i)'s d1 must wait for
            # ACT(i) stage 4 (W0), and A2(i+1) runs during that wait.
            emit_a1(0)
            emit_a2(0)
            for i in range(nt):
                emit_act(i)
                if i + 1 < nt:
                    emit_a1(i + 1)
                    emit_a2(i + 1)
                emit_b(i)
    nc.compile()
    return nc


_NC_CACHE: dict = {}


def _get_nc(rt: int = RT, nt: int = NT):
    key = (rt, nt)
    if key not in _NC_CACHE:
        _NC_CACHE[key] = build_nc(rt, nt)
    return _NC_CACHE[key]


# ---------------------------------------------------------------- host shard


def make_in_maps(a: np.ndarray, b: np.ndarray, rt: int = RT, nt: int = NT):
    """Shard + component-major permute: [B,12] -> 8 x [nt, P, 12*rt]."""
    rows_pc = P * rt * nt
    perm = np.asarray(COMP_PERM, dtype=np.int64)

    def shard(x):
        sh = np.zeros((N_CORES, rows_pc, COLS), dtype=np.float32)
        sh[:, :ROWS_VALID, :] = x.reshape(N_CORES, ROWS_VALID, COLS)[..., perm]
        # [C, nt, P, rt, 12] -> [C, nt, P, 12, rt] -> [C, nt, P, 12*rt]
        sh = sh.reshape(N_CORES, nt, P, rt, COLS)
        sh = np.ascontiguousarray(sh.transpose(0, 1, 2, 4, 3))
        return sh.reshape(N_CORES, nt, P, COLS * rt)

    a_sh = shard(a)
    b_sh = shard(b)
    return [
        {"output": a_sh[c], "target": b_sh[c]} for c in range(N_CORES)
    ]


# ---------------------------------------------------------------- entrypoint


def kernel(output, target):
    a = np.asarray(output, dtype=np.float32)
    b = np.asarray(target, dtype=np.float32)
    assert a.shape == (B, COLS) and b.shape == (B, COLS)

    nc = _get_nc()
    in_maps = make_in_maps(a, b)
    r = run_bass_kernel_spmd(nc, in_maps, list(range(N_CORES)))
    out = np.empty((N_CORES, ROWS_VALID), dtype=np.float32)
    for c in range(N_CORES):
        loss = r.results[c]["loss"].reshape(NT * P * RT)
        out[c] = loss[:ROWS_VALID]
    return out.reshape(B)
